# revision 14
# baseline (speedup 1.0000x reference)
# MoE top-2 routing kernel for Trainium2, 8 NeuronCores, data-parallel over batch.
#
# Dense-expert formulation: every expert matmul is computed for every token tile
# (E=8 is small), combined with masked top-2 softmax weights, then projected.
# No dispatch/scatter/gather and no DRAM scratch round trips.
#
# Host->device traffic is minimized (it dominates wall time on this stack):
#   - expert weights ship expert-sharded in bf16 (each core carries only its
#     own expert slice + a column slice of Wo) and are all-gathered on-device
#   - the output ships back as bf16
# Self-contained: hardcodes shapes B=8, S=2048, D=1024, E=8, TOP_K=2.
import numpy as np
import ml_dtypes

B, S, D, E = 8, 2048, 1024, 8
TOPK = 2
P = 128
NKT = D // P             # 8 contraction tiles
NT = S // P              # 16 token tiles per core
WSH = D + P              # per-core weight shard columns: [WeT_c | WoT col-slice]
XTRA = E + 1 + E + 1     # extra X rows carrying WrT cols, br, be, bo


def build_kernel(s_local=S, n_cores=B):
    import concourse.bacc as bacc
    import concourse.tile as tile
    import concourse.mybir as mybir
    from concourse.masks import make_identity

    dt = mybir.dt
    nt = s_local // P

    nc = bacc.Bacc(None, target_bir_lowering=False, debug=False,
                   num_devices=n_cores)

    # X is extended with XTRA rows carrying the small fp32 tensors
    # (WrT columns, br, be, bo) to minimize the per-call buffer count.
    Xd = nc.declare_dram_parameter("X", [s_local + XTRA, D], dt.float16,
                                   isOutput=False)
    Wshd = nc.declare_dram_parameter("Wsh", [D, WSH], dt.float16, isOutput=False)
    outd = nc.declare_dram_parameter("out", [s_local, D], dt.bfloat16,
                                     isOutput=True)
    # row offsets of the small tensors inside Xd
    R_WRT = s_local            # rows R_WRT..R_WRT+8: WrT column e (length D)
    R_BR = s_local + E         # one row: br in first E cols
    R_BE = s_local + E + 1     # 8 rows: be[e]
    R_BO = s_local + 2 * E + 1  # one row: bo

    fp32 = dt.float32
    bf16 = dt.bfloat16
    fp16 = dt.float16

    # internal DRAM for the weight all-gather
    WshIn = nc.dram_tensor("wsh_in", [D, WSH], fp16)
    WG = nc.dram_tensor("wsh_all", [n_cores, D, WSH], fp16, addr_space="Shared")

    with tile.TileContext(nc) as tc:
        with tc.tile_pool(name="const", bufs=1) as const_p, \
             tc.tile_pool(name="big", bufs=1) as big_p, \
             tc.tile_pool(name="we", bufs=2) as we_p, \
             tc.tile_pool(name="xf", bufs=3) as xf_p, \
             tc.tile_pool(name="xt32", bufs=4) as xt_p, \
             tc.tile_pool(name="sm", bufs=4) as sm_p, \
             tc.tile_pool(name="cmb", bufs=4) as cmb_p, \
             tc.tile_pool(name="oc", bufs=3) as oc_p, \
             tc.tile_pool(name="ps_tr", bufs=2, space="PSUM") as pst_p, \
             tc.tile_pool(name="ps_lg", bufs=2, space="PSUM") as psl_p, \
             tc.tile_pool(name="ps_mm", bufs=2, space="PSUM") as mm_p:

            # ---------- weight all-gather (overlaps router phase below) ----------
            nc.sync.dma_start(out=WshIn[:], in_=Wshd[:])
            nc.gpsimd.collective_compute(
                "AllGather", mybir.AluOpType.bypass,
                replica_groups=[[i for i in range(n_cores)]],
                ins=[WshIn[:]], outs=[WG[:]])

            # ---------- constants ----------
            ID = const_p.tile([P, P], fp32)
            make_identity(nc, ID[:])
            IDb = const_p.tile([P, P], bf16)
            nc.vector.tensor_copy(out=IDb[:], in_=ID[:])
            IDh = const_p.tile([P, P], fp16)
            nc.vector.tensor_copy(out=IDh[:], in_=ID[:])
            # per-expert tie-break bias: -e * 1e-7 (favors lower expert index,
            # matching jax.lax.top_k tie-breaking on equal weights)
            EBi = const_p.tile([P, E], dt.int32)
            nc.gpsimd.iota(EBi[:], pattern=[[1, E]], base=0, channel_multiplier=0)
            EB = const_p.tile([P, E], fp32)
            nc.vector.tensor_scalar_mul(EB[:], EBi[:], -1e-7)
            WrTs = const_p.tile([P, NKT, E], fp16)   # [128, kt, 8]
            for e in range(E):
                nc.sync.dma_start(
                    out=WrTs[:, :, e:e + 1],
                    in_=Xd[R_WRT + e:R_WRT + e + 1, :].rearrange(
                        "o (kt p) -> p kt o", p=P))
            brS = const_p.tile([E, 1], fp32)
            nc.gpsimd.dma_start(
                out=brS[:], in_=Xd[R_BR:R_BR + 1, :E].rearrange("o e -> e o"))
            be9 = const_p.tile([E + 1, D], fp32)
            nc.gpsimd.dma_start(out=be9[:E, :], in_=Xd[R_BE:R_BE + E, :])
            nc.gpsimd.dma_start(out=be9[E:E + 1, :], in_=Xd[R_BO:R_BO + 1, :])
            # WoT assembled from the gathered shards: [128, kt, 1024] bf16
            WoTb = const_p.tile([P, NKT, D], fp16)
            for c in range(n_cores):
                nc.sync.dma_start(
                    out=WoTb[:, :, c * P:(c + 1) * P],
                    in_=WG[c][:, D:].rearrange("(kt p) h -> p kt h", p=P))

            # be9p = [be @ WoT ; bo]  (expert bias folded through output proj)
            beTb = const_p.tile([P, NKT, E + 1], fp16)   # be^T (col E zero), fp16
            nc.vector.memset(beTb[:], 0.0)
            for kt in range(NKT):
                ptb = pst_p.tile([P, E], fp32, space="PSUM", tag="tr")
                nc.tensor.transpose(
                    out=ptb[:], in_=be9[:E, kt * P:(kt + 1) * P],
                    identity=ID[:E, :E])
                nc.vector.tensor_copy(out=beTb[:, kt, :E], in_=ptb[:])
            e9 = const_p.tile([1, E + 1], fp32)
            nc.vector.memset(e9[:], 0.0)
            nc.vector.memset(e9[:, E:], 1.0)
            bo_s = const_p.tile([1, D], fp32)
            nc.gpsimd.dma_start(out=bo_s[:], in_=Xd[R_BO:R_BO + 1, :])
            ps9 = mm_p.tile([E + 1, D], fp32, space="PSUM", tag="mm")
            for h2 in range(2):
                hsl = slice(h2 * 512, (h2 + 1) * 512)
                for kt in range(NKT):
                    nc.tensor.matmul(
                        out=ps9[:, hsl], lhsT=beTb[:, kt, :],
                        rhs=WoTb[:, kt, hsl], start=(kt == 0), stop=False)
                nc.tensor.matmul(
                    out=ps9[:, hsl], lhsT=e9[:], rhs=bo_s[:, hsl],
                    start=False, stop=True)
            be9p = const_p.tile([E + 1, D], fp32)
            nc.vector.tensor_copy(out=be9p[:], in_=ps9[:])

            # ---------- persistent big tiles ----------
            XTb = big_p.tile([P, NKT, s_local], fp16, tag="xtb")   # X^T fp16
            ACC = big_p.tile([P, nt, D], fp16, tag="acc")          # combined
            Wtop2 = big_p.tile([P, nt * E], fp32, tag="wtop2")     # masked top-2 w

            # ---------- phase 1: load X, transpose, router, top-2 ----------
            for t in range(nt):
                xf = xf_p.tile([P, D], fp16)
                nc.sync.dma_start(out=xf[:], in_=Xd[t * P:(t + 1) * P, :])
                lg = psl_p.tile([E, P], fp32, space="PSUM", tag="lg")
                for kt in range(NKT):
                    pt = pst_p.tile([P, P], fp16, space="PSUM", tag="tr")
                    nc.tensor.transpose(
                        out=pt[:], in_=xf[:, kt * P:(kt + 1) * P], identity=IDh[:])
                    xt16 = xt_p.tile([P, P], fp16)
                    nc.vector.tensor_copy(out=xt16[:], in_=pt[:])
                    nc.vector.tensor_copy(
                        out=XTb[:, kt, t * P:(t + 1) * P], in_=pt[:])
                    nc.tensor.matmul(
                        out=lg[:], lhsT=WrTs[:, kt, :], rhs=xt16[:],
                        start=(kt == 0), stop=(kt == NKT - 1))
                # +br on [E, 128] form, then transpose to [128, E]
                lgs = sm_p.tile([E, P], fp32, tag="lgs")
                nc.vector.tensor_scalar(
                    out=lgs[:], in0=lg[:], scalar1=brS[:, 0:1], scalar2=None,
                    op0=mybir.AluOpType.add)
                lp = pst_p.tile([P, E], fp32, space="PSUM", tag="tr")
                nc.tensor.transpose(out=lp[:], in_=lgs[:], identity=ID[:E, :E])
                Ls = sm_p.tile([P, E], fp32, tag="ls")
                nc.vector.tensor_copy(out=Ls[:], in_=lp[:])
                # softmax over 8 experts
                mneg = sm_p.tile([P, 1], fp32, tag="mneg")
                nc.vector.tensor_reduce(
                    out=mneg[:], in_=Ls[:], axis=mybir.AxisListType.X,
                    op=mybir.AluOpType.max, negate=True)
                Eexp = sm_p.tile([P, E], fp32, tag="eexp")
                Zs = sm_p.tile([P, 1], fp32, tag="zs")
                nc.scalar.activation(
                    out=Eexp[:], in_=Ls[:], func=mybir.ActivationFunctionType.Exp,
                    bias=mneg[:, 0:1], scale=1.0, accum_out=Zs[:, 0:1])
                rZ = sm_p.tile([P, 1], fp32, tag="rz")
                nc.vector.reciprocal(out=rZ[:], in_=Zs[:])
                Wsm = sm_p.tile([P, E], fp32, tag="wsm")
                nc.vector.tensor_scalar_mul(Wsm[:], Eexp[:], rZ[:, 0:1])
                nc.vector.tensor_add(out=Wsm[:], in0=Wsm[:], in1=EB[:])
                # top-2 mask: keep top-2 softmax weights, zero the rest
                Wm8 = sm_p.tile([P, E], fp32, tag="wm8")
                nc.vector.max(out=Wm8[:], in_=Wsm[:])
                mr8 = sm_p.tile([P, E], fp32, tag="mr8")
                nc.vector.tensor_copy(out=mr8[:], in_=Wm8[:])
                nc.vector.memset(mr8[:, TOPK:], -1.0)
                Wz = sm_p.tile([P, E], fp32, tag="wz")
                nc.vector.match_replace(
                    out=Wz[:], in_to_replace=mr8[:], in_values=Wsm[:], imm_value=0.0)
                nc.vector.tensor_sub(
                    out=Wtop2[:, t * E:(t + 1) * E], in0=Wsm[:], in1=Wz[:])

            # ---------- phase 2: dense expert matmuls + weighted combine ----------
            for e in range(E):
                web = we_p.tile([P, NKT, D], fp16)   # WeT[e] from the all-gather
                nc.sync.dma_start(
                    out=web[:],
                    in_=WG[e][:, :D].rearrange("(kt p) h -> p kt h", p=P))
                for t in range(nt):
                    zp = mm_p.tile([P, D], fp32, space="PSUM", tag="mm")
                    for kt in range(NKT):
                        for h2 in range(2):
                            nc.tensor.matmul(
                                out=zp[:, h2 * 512:(h2 + 1) * 512],
                                lhsT=XTb[:, kt, t * P:(t + 1) * P],
                                rhs=web[:, kt, h2 * 512:(h2 + 1) * 512],
                                start=(kt == 0), stop=(kt == NKT - 1))
                    wcol = Wtop2[:, t * E + e:t * E + e + 1]
                    if e == 0:
                        nc.vector.tensor_scalar_mul(ACC[:, t, :], zp[:], wcol)
                    else:
                        tmp = cmb_p.tile([P, D], fp16, tag="cmb")
                        nc.scalar.activation(
                            out=tmp[:], in_=zp[:],
                            func=mybir.ActivationFunctionType.Copy, scale=wcol)
                        nc.vector.tensor_add(
                            out=ACC[:, t, :], in0=ACC[:, t, :], in1=tmp[:])

            # ---------- phase 3: output projection + biases ----------
            for t in range(nt):
                accT = oc_p.tile([P, NKT, P], fp16, tag="acct")
                for kt in range(NKT):
                    ptt = pst_p.tile([P, P], fp16, space="PSUM", tag="tr")
                    nc.tensor.transpose(
                        out=ptt[:], in_=ACC[:, t, kt * P:(kt + 1) * P],
                        identity=IDh[:])
                    nc.vector.tensor_copy(out=accT[:, kt, :], in_=ptt[:])
                # W9 = [Wtop2_t | ones] -> transpose -> [9, 128]
                w9 = oc_p.tile([P, E + 1], fp32, tag="w9")
                nc.vector.tensor_copy(out=w9[:, :E], in_=Wtop2[:, t * E:(t + 1) * E])
                nc.vector.memset(w9[:, E:], 1.0)
                w9tp = pst_p.tile([E + 1, P], fp32, space="PSUM", tag="tr")
                nc.tensor.transpose(out=w9tp[:], in_=w9[:], identity=ID[:])
                w9t = oc_p.tile([E + 1, P], fp32, tag="w9t")
                nc.vector.tensor_copy(out=w9t[:], in_=w9tp[:])

                op = mm_p.tile([P, D], fp32, space="PSUM", tag="mm")
                for h2 in range(2):
                    hsl = slice(h2 * 512, (h2 + 1) * 512)
                    for kt in range(NKT):
                        nc.tensor.matmul(
                            out=op[:, hsl], lhsT=accT[:, kt, :],
                            rhs=WoTb[:, kt, hsl], start=(kt == 0), stop=False)
                    nc.tensor.matmul(
                        out=op[:, hsl], lhsT=w9t[:], rhs=be9p[:, hsl],
                        start=False, stop=True)
                osb = oc_p.tile([P, D], bf16, tag="osb")
                if t % 2 == 0:
                    nc.vector.tensor_copy(out=osb[:], in_=op[:])
                else:
                    nc.scalar.activation(
                        out=osb[:], in_=op[:], func=mybir.ActivationFunctionType.Copy)
                nc.sync.dma_start(out=outd[t * P:(t + 1) * P, :], in_=osb[:])

    nc.compile()
    return nc


_NC_CACHE = {}


def _get_nc(s_local=S):
    if s_local not in _NC_CACHE:
        _NC_CACHE[s_local] = build_kernel(s_local)
    return _NC_CACHE[s_local]


def make_in_maps(X, We, be, Wr, br, Wo, bo):
    bf = np.float16
    We = np.asarray(We, np.float32)
    WoT = np.asarray(Wo, np.float32).T            # [d, h]
    Xc = np.asarray(X, np.float32)
    s_local = Xc.shape[1]
    # extra rows appended to each core's X: WrT columns, br, be, bo
    extra = np.zeros((XTRA, D), np.float16)
    extra[:E, :] = np.asarray(Wr, np.float32)     # row e = Wr[e] = WrT[:, e]
    extra[E, :E] = np.asarray(br, np.float32).reshape(E)
    extra[E + 1:2 * E + 1, :] = np.asarray(be, np.float32)
    extra[2 * E + 1, :] = np.asarray(bo, np.float32).reshape(D)
    maps = []
    for c in range(B):
        xe = np.empty((s_local + XTRA, D), np.float16)
        xe[:s_local] = Xc[c]
        xe[s_local:] = extra
        wsh = np.empty((D, WSH), bf)
        wsh[:, :D] = We[c].T.astype(bf)           # WeT_c [d, h]
        wsh[:, D:] = WoT[:, c * P:(c + 1) * P].astype(bf)
        maps.append({"X": xe, "Wsh": wsh})
    return maps


def kernel(X, We, be, Wr, br, Wo, bo):
    from concourse.bass_utils import run_bass_kernel_spmd
    nc = _get_nc()
    in_maps = make_in_maps(X, We, be, Wr, br, Wo, bo)
    res = run_bass_kernel_spmd(nc, in_maps, list(range(B)))
    out = np.stack([res.results[c]["out"] for c in range(B)], axis=0)
    return out.astype(np.float32)


# revision 19
# speedup vs baseline: 1.0036x; 1.0036x over previous
# MoE top-2 routing kernel for Trainium2, 8 NeuronCores, data-parallel over batch.
#
# Dense-expert formulation: every expert matmul is computed for every token tile
# (E=8 is small), combined with masked top-2 softmax weights, then projected.
# No dispatch/scatter/gather and no DRAM scratch round trips.
#
# Host->device traffic is minimized (it dominates wall time on this stack):
#   - expert weights ship expert-sharded in bf16 (each core carries only its
#     own expert slice + a column slice of Wo) and are all-gathered on-device
#   - the output ships back as bf16
# Self-contained: hardcodes shapes B=8, S=2048, D=1024, E=8, TOP_K=2.
import numpy as np
import ml_dtypes

B, S, D, E = 8, 2048, 1024, 8
TOPK = 2
P = 128
NKT = D // P             # 8 contraction tiles
NT = S // P              # 16 token tiles per core
WSH = D + P              # per-core weight shard columns: [WeT_c | WoT col-slice]
XTRA = E + 1 + E + 1     # extra X rows carrying WrT cols, br, be, bo
WSH_ROWS = (D * WSH) // D  # Wsh flattened into D-wide rows appended to X


def build_kernel(s_local=S, n_cores=B):
    import concourse.bacc as bacc
    import concourse.tile as tile
    import concourse.mybir as mybir
    from concourse.masks import make_identity

    dt = mybir.dt
    nt = s_local // P

    nc = bacc.Bacc(None, target_bir_lowering=False, debug=False,
                   num_devices=n_cores)

    # X is extended with rows carrying the small tensors (WrT columns, br,
    # be, bo) and the flattened weight shard, so the whole per-core payload
    # is a single staged buffer.
    Xd = nc.declare_dram_parameter("X", [s_local + XTRA + WSH_ROWS, D],
                                   dt.float16, isOutput=False)
    outd = nc.declare_dram_parameter("out", [s_local, D], dt.bfloat16,
                                     isOutput=True)
    # row offsets of the small tensors inside Xd
    R_WRT = s_local            # rows R_WRT..R_WRT+8: WrT column e (length D)
    R_BR = s_local + E         # one row: br in first E cols
    R_BE = s_local + E + 1     # 8 rows: be[e]
    R_BO = s_local + 2 * E + 1  # one row: bo
    R_WSH = s_local + XTRA     # WSH_ROWS rows: Wsh [D, WSH] flattened

    fp32 = dt.float32
    bf16 = dt.bfloat16
    fp16 = dt.float16

    # internal DRAM for the weight all-gathers (WoT slice gathered separately
    # and first: it is small and unblocks the const/bias prep early)
    WeIn = nc.dram_tensor("we_in", [D, D], fp16)
    WoIn = nc.dram_tensor("wo_in", [D, P], fp16)
    WGe = nc.dram_tensor("we_all", [n_cores, D, D], fp16, addr_space="Shared")
    WGo = nc.dram_tensor("wo_all", [n_cores, D, P], fp16, addr_space="Shared")

    with tile.TileContext(nc) as tc:
        with tc.tile_pool(name="const", bufs=1) as const_p, \
             tc.tile_pool(name="big", bufs=1) as big_p, \
             tc.tile_pool(name="we", bufs=2) as we_p, \
             tc.tile_pool(name="xf", bufs=3) as xf_p, \
             tc.tile_pool(name="xt32", bufs=4) as xt_p, \
             tc.tile_pool(name="sm", bufs=4) as sm_p, \
             tc.tile_pool(name="cmb", bufs=4) as cmb_p, \
             tc.tile_pool(name="oc", bufs=3) as oc_p, \
             tc.tile_pool(name="ps_tr", bufs=2, space="PSUM") as pst_p, \
             tc.tile_pool(name="ps_lg", bufs=2, space="PSUM") as psl_p, \
             tc.tile_pool(name="ps_mm", bufs=2, space="PSUM") as mm_p:

            # ---------- weight all-gathers (overlap router phase below) ----------
            # view of the flattened Wsh rows inside Xd as [D, WSH]
            WshAP = Xd[R_WSH:R_WSH + WSH_ROWS, :].rearrange(
                "a b -> (a b)").rearrange("(d w) -> d w", w=WSH)
            nc.sync.dma_start(out=WoIn[:], in_=WshAP[:, D:])
            nc.sync.dma_start(out=WeIn[:], in_=WshAP[:, :D])
            groups = [[i for i in range(n_cores)]]
            nc.gpsimd.collective_compute(
                "AllGather", mybir.AluOpType.bypass, replica_groups=groups,
                ins=[WoIn[:]], outs=[WGo[:]])
            nc.gpsimd.collective_compute(
                "AllGather", mybir.AluOpType.bypass, replica_groups=groups,
                ins=[WeIn[:]], outs=[WGe[:]])

            # ---------- constants ----------
            ID = const_p.tile([P, P], fp32)
            make_identity(nc, ID[:])
            IDb = const_p.tile([P, P], bf16)
            nc.vector.tensor_copy(out=IDb[:], in_=ID[:])
            IDh = const_p.tile([P, P], fp16)
            nc.vector.tensor_copy(out=IDh[:], in_=ID[:])
            # per-expert tie-break bias: -e * 1e-7 (favors lower expert index,
            # matching jax.lax.top_k tie-breaking on equal weights)
            EBi = const_p.tile([P, E], dt.int32)
            nc.gpsimd.iota(EBi[:], pattern=[[1, E]], base=0, channel_multiplier=0)
            EB = const_p.tile([P, E], fp32)
            nc.vector.tensor_scalar_mul(EB[:], EBi[:], -1e-7)
            WrTs = const_p.tile([P, NKT, E], fp16)   # [128, kt, 8]
            for e in range(E):
                nc.sync.dma_start(
                    out=WrTs[:, :, e:e + 1],
                    in_=Xd[R_WRT + e:R_WRT + e + 1, :].rearrange(
                        "o (kt p) -> p kt o", p=P))
            brS = const_p.tile([E, 1], fp32)
            nc.gpsimd.dma_start(
                out=brS[:], in_=Xd[R_BR:R_BR + 1, :E].rearrange("o e -> e o"))
            be9 = const_p.tile([E + 1, D], fp32)
            nc.gpsimd.dma_start(out=be9[:E, :], in_=Xd[R_BE:R_BE + E, :])
            nc.gpsimd.dma_start(out=be9[E:E + 1, :], in_=Xd[R_BO:R_BO + 1, :])
            # WoT assembled from the gathered shards: [128, kt, 1024] fp16
            WoTb = const_p.tile([P, NKT, D], fp16)
            for c in range(n_cores):
                nc.sync.dma_start(
                    out=WoTb[:, :, c * P:(c + 1) * P],
                    in_=WGo[c].rearrange("(kt p) h -> p kt h", p=P))

            # be9p = [be @ WoT ; bo]  (expert bias folded through output proj)
            beTb = const_p.tile([P, NKT, E + 1], fp16)   # be^T (col E zero), fp16
            nc.vector.memset(beTb[:], 0.0)
            for kt in range(NKT):
                ptb = pst_p.tile([P, E], fp32, space="PSUM", tag="tr")
                nc.tensor.transpose(
                    out=ptb[:], in_=be9[:E, kt * P:(kt + 1) * P],
                    identity=ID[:E, :E])
                nc.vector.tensor_copy(out=beTb[:, kt, :E], in_=ptb[:])
            e9 = const_p.tile([1, E + 1], fp32)
            nc.vector.memset(e9[:], 0.0)
            nc.vector.memset(e9[:, E:], 1.0)
            bo_s = const_p.tile([1, D], fp32)
            nc.gpsimd.dma_start(out=bo_s[:], in_=Xd[R_BO:R_BO + 1, :])
            ps9 = mm_p.tile([E + 1, D], fp32, space="PSUM", tag="mm")
            for h2 in range(2):
                hsl = slice(h2 * 512, (h2 + 1) * 512)
                for kt in range(NKT):
                    nc.tensor.matmul(
                        out=ps9[:, hsl], lhsT=beTb[:, kt, :],
                        rhs=WoTb[:, kt, hsl], start=(kt == 0), stop=False)
                nc.tensor.matmul(
                    out=ps9[:, hsl], lhsT=e9[:], rhs=bo_s[:, hsl],
                    start=False, stop=True)
            be9p = const_p.tile([E + 1, D], fp32)
            nc.vector.tensor_copy(out=be9p[:], in_=ps9[:])

            # ---------- persistent big tiles ----------
            XTb = big_p.tile([P, NKT, s_local], fp16, tag="xtb")   # X^T fp16
            ACC = big_p.tile([P, nt, D], fp16, tag="acc")          # combined
            Wtop2 = big_p.tile([P, nt * E], fp32, tag="wtop2")     # masked top-2 w

            # ---------- phase 1: load X, transpose, router, top-2 ----------
            for t in range(nt):
                xf = xf_p.tile([P, D], fp16)
                nc.sync.dma_start(out=xf[:], in_=Xd[t * P:(t + 1) * P, :])
                lg = psl_p.tile([E, P], fp32, space="PSUM", tag="lg")
                for kt in range(NKT):
                    pt = pst_p.tile([P, P], fp16, space="PSUM", tag="tr")
                    nc.tensor.transpose(
                        out=pt[:], in_=xf[:, kt * P:(kt + 1) * P], identity=IDh[:])
                    xt16 = xt_p.tile([P, P], fp16)
                    nc.vector.tensor_copy(out=xt16[:], in_=pt[:])
                    nc.vector.tensor_copy(
                        out=XTb[:, kt, t * P:(t + 1) * P], in_=pt[:])
                    nc.tensor.matmul(
                        out=lg[:], lhsT=WrTs[:, kt, :], rhs=xt16[:],
                        start=(kt == 0), stop=(kt == NKT - 1))
                # +br on [E, 128] form, then transpose to [128, E]
                lgs = sm_p.tile([E, P], fp32, tag="lgs")
                nc.vector.tensor_scalar(
                    out=lgs[:], in0=lg[:], scalar1=brS[:, 0:1], scalar2=None,
                    op0=mybir.AluOpType.add)
                lp = pst_p.tile([P, E], fp32, space="PSUM", tag="tr")
                nc.tensor.transpose(out=lp[:], in_=lgs[:], identity=ID[:E, :E])
                Ls = sm_p.tile([P, E], fp32, tag="ls")
                nc.vector.tensor_copy(out=Ls[:], in_=lp[:])
                # softmax over 8 experts
                mneg = sm_p.tile([P, 1], fp32, tag="mneg")
                nc.vector.tensor_reduce(
                    out=mneg[:], in_=Ls[:], axis=mybir.AxisListType.X,
                    op=mybir.AluOpType.max, negate=True)
                Eexp = sm_p.tile([P, E], fp32, tag="eexp")
                Zs = sm_p.tile([P, 1], fp32, tag="zs")
                nc.scalar.activation(
                    out=Eexp[:], in_=Ls[:], func=mybir.ActivationFunctionType.Exp,
                    bias=mneg[:, 0:1], scale=1.0, accum_out=Zs[:, 0:1])
                rZ = sm_p.tile([P, 1], fp32, tag="rz")
                nc.vector.reciprocal(out=rZ[:], in_=Zs[:])
                Wsm = sm_p.tile([P, E], fp32, tag="wsm")
                nc.vector.tensor_scalar_mul(Wsm[:], Eexp[:], rZ[:, 0:1])
                nc.vector.tensor_add(out=Wsm[:], in0=Wsm[:], in1=EB[:])
                # top-2 mask: keep top-2 softmax weights, zero the rest
                Wm8 = sm_p.tile([P, E], fp32, tag="wm8")
                nc.vector.max(out=Wm8[:], in_=Wsm[:])
                mr8 = sm_p.tile([P, E], fp32, tag="mr8")
                nc.vector.tensor_copy(out=mr8[:], in_=Wm8[:])
                nc.vector.memset(mr8[:, TOPK:], -1.0)
                Wz = sm_p.tile([P, E], fp32, tag="wz")
                nc.vector.match_replace(
                    out=Wz[:], in_to_replace=mr8[:], in_values=Wsm[:], imm_value=0.0)
                nc.vector.tensor_sub(
                    out=Wtop2[:, t * E:(t + 1) * E], in0=Wsm[:], in1=Wz[:])

            # ---------- phase 2: dense expert matmuls + weighted combine ----------
            for e in range(E):
                web = we_p.tile([P, NKT, D], fp16)   # WeT[e] from the all-gather
                nc.sync.dma_start(
                    out=web[:],
                    in_=WGe[e].rearrange("(kt p) h -> p kt h", p=P))
                for t in range(nt):
                    zp = mm_p.tile([P, D], fp32, space="PSUM", tag="mm")
                    for kt in range(NKT):
                        for h2 in range(2):
                            nc.tensor.matmul(
                                out=zp[:, h2 * 512:(h2 + 1) * 512],
                                lhsT=XTb[:, kt, t * P:(t + 1) * P],
                                rhs=web[:, kt, h2 * 512:(h2 + 1) * 512],
                                start=(kt == 0), stop=(kt == NKT - 1))
                    wcol = Wtop2[:, t * E + e:t * E + e + 1]
                    if e == 0:
                        nc.vector.tensor_scalar_mul(ACC[:, t, :], zp[:], wcol)
                    else:
                        tmp = cmb_p.tile([P, D], fp16, tag="cmb")
                        nc.scalar.activation(
                            out=tmp[:], in_=zp[:],
                            func=mybir.ActivationFunctionType.Copy, scale=wcol)
                        nc.vector.tensor_add(
                            out=ACC[:, t, :], in0=ACC[:, t, :], in1=tmp[:])

            # ---------- phase 3: output projection + biases ----------
            for t in range(nt):
                accT = oc_p.tile([P, NKT, P], fp16, tag="acct")
                for kt in range(NKT):
                    ptt = pst_p.tile([P, P], fp16, space="PSUM", tag="tr")
                    nc.tensor.transpose(
                        out=ptt[:], in_=ACC[:, t, kt * P:(kt + 1) * P],
                        identity=IDh[:])
                    nc.vector.tensor_copy(out=accT[:, kt, :], in_=ptt[:])
                # W9 = [Wtop2_t | ones] -> transpose -> [9, 128]
                w9 = oc_p.tile([P, E + 1], fp32, tag="w9")
                nc.vector.tensor_copy(out=w9[:, :E], in_=Wtop2[:, t * E:(t + 1) * E])
                nc.vector.memset(w9[:, E:], 1.0)
                w9tp = pst_p.tile([E + 1, P], fp32, space="PSUM", tag="tr")
                nc.tensor.transpose(out=w9tp[:], in_=w9[:], identity=ID[:])
                w9t = oc_p.tile([E + 1, P], fp32, tag="w9t")
                nc.vector.tensor_copy(out=w9t[:], in_=w9tp[:])

                op = mm_p.tile([P, D], fp32, space="PSUM", tag="mm")
                for h2 in range(2):
                    hsl = slice(h2 * 512, (h2 + 1) * 512)
                    for kt in range(NKT):
                        nc.tensor.matmul(
                            out=op[:, hsl], lhsT=accT[:, kt, :],
                            rhs=WoTb[:, kt, hsl], start=(kt == 0), stop=False)
                    nc.tensor.matmul(
                        out=op[:, hsl], lhsT=w9t[:], rhs=be9p[:, hsl],
                        start=False, stop=True)
                osb = oc_p.tile([P, D], bf16, tag="osb")
                if t % 2 == 0:
                    nc.vector.tensor_copy(out=osb[:], in_=op[:])
                else:
                    nc.scalar.activation(
                        out=osb[:], in_=op[:], func=mybir.ActivationFunctionType.Copy)
                nc.sync.dma_start(out=outd[t * P:(t + 1) * P, :], in_=osb[:])

    nc.compile()
    return nc


_NC_CACHE = {}


def _get_nc(s_local=S):
    if s_local not in _NC_CACHE:
        _NC_CACHE[s_local] = build_kernel(s_local)
    return _NC_CACHE[s_local]


def make_in_maps(X, We, be, Wr, br, Wo, bo):
    bf = np.float16
    We = np.asarray(We, np.float32)
    WoT = np.asarray(Wo, np.float32).T            # [d, h]
    Xc = np.asarray(X, np.float32)
    s_local = Xc.shape[1]
    # extra rows appended to each core's X: WrT columns, br, be, bo
    extra = np.zeros((XTRA, D), np.float16)
    extra[:E, :] = np.asarray(Wr, np.float32)     # row e = Wr[e] = WrT[:, e]
    extra[E, :E] = np.asarray(br, np.float32).reshape(E)
    extra[E + 1:2 * E + 1, :] = np.asarray(be, np.float32)
    extra[2 * E + 1, :] = np.asarray(bo, np.float32).reshape(D)
    maps = []
    for c in range(B):
        xe = np.empty((s_local + XTRA + WSH_ROWS, D), np.float16)
        xe[:s_local] = Xc[c]
        xe[s_local:s_local + XTRA] = extra
        wsh = np.empty((D, WSH), bf)
        wsh[:, :D] = We[c].T.astype(bf)           # WeT_c [d, h]
        wsh[:, D:] = WoT[:, c * P:(c + 1) * P].astype(bf)
        xe[s_local + XTRA:] = wsh.reshape(WSH_ROWS, D)
        maps.append({"X": xe})
    return maps


def kernel(X, We, be, Wr, br, Wo, bo):
    from concourse.bass_utils import run_bass_kernel_spmd
    nc = _get_nc()
    in_maps = make_in_maps(X, We, be, Wr, br, Wo, bo)
    res = run_bass_kernel_spmd(nc, in_maps, list(range(B)))
    out = np.stack([res.results[c]["out"] for c in range(B)], axis=0)
    return out.astype(np.float32)
